# revision 4
# baseline (speedup 1.0000x reference)
"""LRU (complex diagonal linear recurrence, fwd+bwd) on 8 TRN2 NeuronCores.

Algorithm (validated in numpy): sequence-parallel over T. Per core:
  Bu^T = B_norm @ x_chunk^T  (fp32r matmuls)
  rotation trick: w = e^{-i*theta*tau} (.) Bu  -> complex scan becomes two
  real first-order scans with multiplier r (hardware tensor_tensor_scan)
  cross-core carries via AllGather of chunk-end states; correction applied
  in v-space as a single scalar_tensor_tensor per component (real decay)
  s = e^{+i*theta*tau} (.) v ;  y^T = C-projections (fp16 matmuls) + D (.) x^T
Backward direction = same machinery on the time-reversed stream.
Host does all transposes/table precompute (free); device does all O(T*N) work.
"""

import numpy as np
from contextlib import ExitStack

import concourse.bass as bass
import concourse.tile as tile
from concourse import bacc, mybir
from concourse.bass_utils import run_bass_kernel_spmd

NCORES = 8
T, N, H = 16384, 512, 512
TC = T // NCORES          # 2048 timesteps per core
NT = N // 128             # 4 partition tiles of the state dim
HT = H // 128             # 4 partition tiles of the channel dim
KH = H // 128             # contraction subtiles for Bu matmul
F16 = mybir.dt.float16
F32 = mybir.dt.float32
F32R = mybir.dt.float32r
MUL = mybir.AluOpType.mult
ADD = mybir.AluOpType.add
SUB = mybir.AluOpType.subtract

_CACHE = {}


def _build_nc(profile=False):
    nc = bacc.Bacc(
        "TRN2", target_bir_lowering=False, debug=False,
        enable_asserts=False, num_devices=1 if profile else NCORES,
    )
    di = lambda n, s, d=F32: nc.dram_tensor(n, s, d, kind="ExternalInput")
    xT_d = di("xT", [H, TC], F16)
    BTre_d = di("BTre", [H, N], F16)
    BTim_d = di("BTim", [H, N], F16)
    cos_d = di("cosT", [N, TC], F16)
    sin_d = di("sinT", [N, TC], F16)
    rpw_d = di("rpow", [N, TC], F16)
    # consts columns: 0=r 1=ce 2=se 3=c1 4=s1 5=D
    cst_d = di("consts", [N, 8])
    CT_d = {(d_, c_): di(f"CT{d_}{c_}", [N, H], F16)
            for d_ in "fb" for c_ in "ri"}
    W_d = {(d_, c_): di(f"W{d_}{c_}", [N, 8]) for d_ in "fb" for c_ in "ri"}
    yT_d = nc.dram_tensor("yT", [H, TC], F32, kind="ExternalOutput")
    bin_d = nc.dram_tensor("ccin", [128, 16], F32)
    bout_d = nc.dram_tensor("ccout", [NCORES, 128, 16], F32)

    with tile.TileContext(nc) as tc, ExitStack() as ctx:
        pool = lambda name, bufs: ctx.enter_context(tc.tile_pool(name=name, bufs=bufs))
        p_xT = pool("xT", 4)
        p_BT = pool("BT", 8)
        p_tab = pool("tab", 4)          # cos/sin, transient per nt per phase
        p_rpw = pool("rpw", 2)
        p_cst = pool("cst", 4)
        p_CT = pool("CT", 16)
        p_bups = ctx.enter_context(tc.tile_pool(name="bups", bufs=2, space="PSUM"))
        p_bu16 = pool("bu16", 3)
        p_w = pool("w", 4)
        p_st = pool("st", 22)           # v tiles, s-hat tiles, rotation temps
        p_sm = pool("sm", 24)           # small (128,<=16) helpers
        p_ops = ctx.enter_context(tc.tile_pool(name="ops", bufs=3, space="PSUM"))
        p_yo = pool("yo", 3)

        # ---- resident loads ----
        xT_sb = []
        for h in range(HT):
            t_ = p_xT.tile([128, TC], F16, tag="xT")
            nc.sync.dma_start(t_[:], xT_d[h * 128:(h + 1) * 128, :])
            xT_sb.append(t_)
        BT_sb = {}
        for nm, dd in (("re", BTre_d), ("im", BTim_d)):
            for h in range(HT):
                t_ = p_BT.tile([128, N], F16, tag="BT")
                nc.sync.dma_start(t_[:], dd[h * 128:(h + 1) * 128, :])
                BT_sb[(nm, h)] = t_
        cst_sb = []
        for nt in range(NT):
            t_ = p_cst.tile([128, 8], F32, tag="cst")
            nc.sync.dma_start(t_[:], cst_d[nt * 128:(nt + 1) * 128, :])
            cst_sb.append(t_)
        CT_sb = {}
        for key, dd in CT_d.items():
            for nt in range(NT):
                t_ = p_CT.tile([128, H], F16, tag="CT")
                nc.sync.dma_start(t_[:], dd[nt * 128:(nt + 1) * 128, :])
                CT_sb[key + (nt,)] = t_
        W_sb = {}
        for key, dd in W_d.items():
            for nt in range(NT):
                t_ = p_sm.tile([128, 8], F32, tag="sm")
                nc.sync.dma_start(t_[:], dd[nt * 128:(nt + 1) * 128, :])
                W_sb[key + (nt,)] = t_

        # ---- per N-tile: Bu matmuls, pre-rotations, pass-1 scans ----
        v_sb = {}      # (nt, dir, comp) -> fp16 (128, TC) local-scan outputs
        epk = p_sm.tile([128, 16], F32, tag="epk")   # packed end states
        for nt in range(NT):
            cos_t = p_tab.tile([128, TC], F16, tag="tab")
            nc.sync.dma_start(cos_t[:], cos_d[nt * 128:(nt + 1) * 128, :])
            sin_t = p_tab.tile([128, TC], F16, tag="tab")
            nc.sync.dma_start(sin_t[:], sin_d[nt * 128:(nt + 1) * 128, :])
            bu16 = {}
            for ci, nm in enumerate(("re", "im")):
                bu = p_bu16.tile([128, TC], F16, tag="bu16")
                for half in range(2):
                    ps = p_bups.tile([128, TC // 2], F32, tag="bups")
                    for lc in range(2):
                        sl = slice(half * 1024 + lc * 512, half * 1024 + (lc + 1) * 512)
                        psl = slice(lc * 512, (lc + 1) * 512)
                        for kh in range(KH):
                            nc.tensor.matmul(
                                ps[:, psl],
                                BT_sb[(nm, kh)][:, nt * 128:(nt + 1) * 128],
                                xT_sb[kh][:, sl],
                                start=(kh == 0), stop=(kh == KH - 1),
                            )
                    nc.scalar.copy(bu[:, half * 1024:(half + 1) * 1024], ps[:])
                bu16[nm] = bu
            rbc = cst_sb[nt][:, 0:1].broadcast_to([128, TC])
            for d_ in "fb":
                if d_ == "f":
                    a = bu16["re"][:]; b = bu16["im"][:]
                else:
                    a = bu16["re"][:, ::-1]; b = bu16["im"][:, ::-1]
                t1 = p_st.tile([128, TC], F16, tag="st")
                t2 = p_st.tile([128, TC], F16, tag="st")
                nc.vector.tensor_tensor(t1[:], cos_t[:], a, MUL)
                nc.vector.tensor_tensor(t2[:], sin_t[:], b, MUL)
                w_re = p_w.tile([128, TC], F16, tag="w")
                nc.vector.tensor_tensor(w_re[:], t1[:], t2[:], ADD)
                nc.vector.tensor_tensor(t1[:], cos_t[:], b, MUL)
                nc.vector.tensor_tensor(t2[:], sin_t[:], a, MUL)
                w_im = p_w.tile([128, TC], F16, tag="w")
                nc.vector.tensor_tensor(w_im[:], t1[:], t2[:], SUB)
                for ci, wt in (("re", w_re), ("im", w_im)):
                    v = p_st.tile([128, TC], F16, tag="st")
                    nc.vector.tensor_tensor_scan(v[:], rbc, wt[:], 0.0, MUL, ADD)
                    v_sb[(nt, d_, ci)] = v
                # end states -> s-space: E = (ce + i*se) * v_end
                ce = cst_sb[nt][:, 1:2]; se = cst_sb[nt][:, 2:3]
                vre = v_sb[(nt, d_, "re")][:, TC - 1:TC]
                vim = v_sb[(nt, d_, "im")][:, TC - 1:TC]
                tt = p_sm.tile([128, 1], F32, tag="sm")
                col = (0 if d_ == "f" else 8) + nt * 2
                nc.vector.tensor_scalar_mul(tt[:], vim, se)
                nc.vector.scalar_tensor_tensor(epk[:, col:col + 1], vre, ce, tt[:], MUL, SUB)
                nc.vector.tensor_scalar_mul(tt[:], vre, se)
                nc.vector.scalar_tensor_tensor(epk[:, col + 1:col + 2], vim, ce, tt[:], MUL, ADD)

        # ---- carry exchange ----
        nc.sync.dma_start(bin_d[:, :], epk[:])
        if profile:
            # TimelineSim can't model collectives; stand in a same-cost DMA
            for j in range(NCORES):
                nc.sync.dma_start(bout_d.ap()[j, :, :], bin_d[:, :])
        else:
            nc.gpsimd.collective_compute(
                "AllGather", mybir.AluOpType.bypass,
                replica_groups=[list(range(NCORES))],
                ins=[bin_d.ap().opt()], outs=[bout_d.ap().opt()],
            )
        chv = {}
        for d_ in "fb":
            for nt in range(NT):
                col = (0 if d_ == "f" else 8) + nt * 2
                eg = p_sm.tile([128, 16], F32, tag="eg")
                nc.sync.dma_start(
                    eg[:].rearrange("p (j c) -> p j c", c=2),
                    bout_d.ap()[:, :, col:col + 2].rearrange("j p c -> p j c"),
                )
                er = eg[:, 0:16:2]; ei = eg[:, 1:16:2]
                wre = W_sb[(d_, "r", nt)][:]; wim = W_sb[(d_, "i", nt)][:]
                pr = p_sm.tile([128, 8], F32, tag="pr")
                pi = p_sm.tile([128, 8], F32, tag="pr")
                cre = p_sm.tile([128, 1], F32, tag="cc")
                cim = p_sm.tile([128, 1], F32, tag="cc")
                nc.vector.tensor_tensor(pr[:], wre, er, MUL)
                nc.vector.tensor_tensor(pi[:], wim, ei, MUL)
                nc.vector.tensor_tensor(pr[:], pr[:], pi[:], SUB)
                nc.vector.tensor_reduce(cre[:], pr[:], mybir.AxisListType.X, ADD)
                nc.vector.tensor_tensor(pr[:], wre, ei, MUL)
                nc.vector.tensor_tensor(pi[:], wim, er, MUL)
                nc.vector.tensor_tensor(pr[:], pr[:], pi[:], ADD)
                nc.vector.tensor_reduce(cim[:], pr[:], mybir.AxisListType.X, ADD)
                # chv = e^{i theta} * c
                c1 = cst_sb[nt][:, 3:4]; s1 = cst_sb[nt][:, 4:5]
                tt = p_sm.tile([128, 1], F32, tag="sm")
                vr = p_sm.tile([128, 1], F32, tag="cv")
                vi = p_sm.tile([128, 1], F32, tag="cv")
                nc.vector.tensor_scalar_mul(tt[:], cim[:], s1)
                nc.vector.scalar_tensor_tensor(vr[:], cre[:], c1, tt[:], MUL, SUB)
                nc.vector.tensor_scalar_mul(tt[:], cre[:], s1)
                nc.vector.scalar_tensor_tensor(vi[:], cim[:], c1, tt[:], MUL, ADD)
                chv[(nt, d_, "re")] = vr
                chv[(nt, d_, "im")] = vi

        # ---- corrections + post-rotations ----
        sh_sb = {}
        for nt in range(NT):
            rpw = p_rpw.tile([128, TC], F16, tag="rpw")
            nc.sync.dma_start(rpw[:], rpw_d[nt * 128:(nt + 1) * 128, :])
            cos_t = p_tab.tile([128, TC], F16, tag="tab")
            nc.sync.dma_start(cos_t[:], cos_d[nt * 128:(nt + 1) * 128, :])
            sin_t = p_tab.tile([128, TC], F16, tag="tab")
            nc.sync.dma_start(sin_t[:], sin_d[nt * 128:(nt + 1) * 128, :])
            for d_ in "fb":
                vt = {}
                for ci in ("re", "im"):
                    v2 = p_st.tile([128, TC], F16, tag="st")
                    nc.vector.scalar_tensor_tensor(
                        v2[:], rpw[:], chv[(nt, d_, ci)][:],
                        v_sb[(nt, d_, ci)][:], MUL, ADD)
                    vt[ci] = v2
                t1 = p_st.tile([128, TC], F16, tag="st")
                t2 = p_st.tile([128, TC], F16, tag="st")
                s_re = p_st.tile([128, TC], F16, tag="st")
                s_im = p_st.tile([128, TC], F16, tag="st")
                nc.vector.tensor_tensor(t1[:], sin_t[:], vt["re"][:], MUL)
                nc.vector.tensor_tensor(t2[:], cos_t[:], vt["im"][:], MUL)
                nc.vector.tensor_tensor(s_im[:] if d_ == "f" else s_im[:, ::-1],
                                        t1[:], t2[:], ADD)
                nc.vector.tensor_tensor(t1[:], cos_t[:], vt["re"][:], MUL)
                nc.vector.tensor_tensor(t2[:], sin_t[:], vt["im"][:], MUL)
                nc.vector.tensor_tensor(s_re[:] if d_ == "f" else s_re[:, ::-1],
                                        t1[:], t2[:], SUB)
                sh_sb[(nt, d_, "re")] = s_re
                sh_sb[(nt, d_, "im")] = s_im

        # ---- output matmuls + D term ----
        for lc in range(4):
            lsl = slice(lc * 512, (lc + 1) * 512)
            for ht in range(HT):
                ps = p_ops.tile([128, 512], F32, tag="ops")
                groups = [(d_, c_, nt) for d_ in "fb" for c_ in "ri"
                          for nt in range(NT)]
                for gi, (d_, c_, nt) in enumerate(groups):
                    nc.tensor.matmul(
                        ps[:],
                        CT_sb[(d_, c_, nt)][:, ht * 128:(ht + 1) * 128],
                        sh_sb[(nt, d_, "re" if c_ == "r" else "im")][:, lsl],
                        start=(gi == 0), stop=(gi == len(groups) - 1),
                    )
                yo = p_yo.tile([128, 512], F32, tag="yo")
                nc.vector.scalar_tensor_tensor(
                    yo[:], xT_sb[ht][:, lsl], cst_sb[ht][:, 5:6], ps[:], MUL, ADD)
                nc.sync.dma_start(yT_d[ht * 128:(ht + 1) * 128, lsl], yo[:])

    nc.compile()
    return nc


def _host_prep(x, theta_log, nu_log, B_re, B_im, C_re, C_im, C_re2, C_im2, D):
    f64 = np.float64
    theta = np.exp(theta_log.astype(f64))
    r = np.exp(-np.exp(nu_log.astype(f64)))
    gamma = np.sqrt(1.0 - r ** 2)
    Bn = (B_re.astype(f64) + 1j * B_im.astype(f64)) * gamma[:, None]
    Lam = r * np.exp(1j * theta)
    tau = np.arange(TC, dtype=f64)
    cosT = np.cos(theta[:, None] * tau).astype(np.float16)
    sinT = np.sin(theta[:, None] * tau).astype(np.float16)
    rpow = (r[:, None] ** (tau + 1)).astype(np.float16)
    consts = np.zeros((N, 8), np.float32)
    consts[:, 0] = r
    consts[:, 1] = np.cos(theta * (TC - 1)); consts[:, 2] = np.sin(theta * (TC - 1))
    consts[:, 3] = np.cos(theta); consts[:, 4] = np.sin(theta)
    consts[:, 5] = D
    xT = np.ascontiguousarray(x.T.astype(np.float16))        # (H, T)
    BTre = np.ascontiguousarray(Bn.real.T.astype(np.float16))
    BTim = np.ascontiguousarray(Bn.imag.T.astype(np.float16))
    C1 = C_re.astype(f64) + 1j * C_im.astype(f64)
    C2 = C_re2.astype(f64) + 1j * C_im2.astype(f64)
    CT = {
        ("f", "r"): C1.real.T, ("f", "i"): -C1.imag.T,
        ("b", "r"): C2.real.T, ("b", "i"): -C2.imag.T,
    }
    CT = {k: np.ascontiguousarray(v.astype(np.float16)) for k, v in CT.items()}
    LamTC = Lam ** TC
    W = {}
    for k in range(NCORES):
        wf = np.zeros((N, 8), np.complex128)
        wb = np.zeros((N, 8), np.complex128)
        for j in range(k):
            wf[:, j] = LamTC ** (k - 1 - j)
        for j in range(k + 1, NCORES):
            wb[:, j] = LamTC ** (j - k - 1)
        W[k] = (wf, wb)
    return xT, BTre, BTim, cosT, sinT, rpow, consts, CT, W


def kernel(**inputs):
    if "nc" not in _CACHE:
        _CACHE["nc"] = _build_nc()
    nc = _CACHE["nc"]
    xT, BTre, BTim, cosT, sinT, rpow, consts, CT, W = _host_prep(**inputs)
    in_maps = []
    for k in range(NCORES):
        wf, wb = W[k]
        m = {
            "xT": np.ascontiguousarray(xT[:, k * TC:(k + 1) * TC]),
            "BTre": BTre, "BTim": BTim,
            "cosT": cosT, "sinT": sinT, "rpow": rpow, "consts": consts,
            "CTfr": CT[("f", "r")], "CTfi": CT[("f", "i")],
            "CTbr": CT[("b", "r")], "CTbi": CT[("b", "i")],
            "Wfr": np.ascontiguousarray(wf.real.astype(np.float32)),
            "Wfi": np.ascontiguousarray(wf.imag.astype(np.float32)),
            "Wbr": np.ascontiguousarray(wb.real.astype(np.float32)),
            "Wbi": np.ascontiguousarray(wb.imag.astype(np.float32)),
        }
        in_maps.append(m)
    res = run_bass_kernel_spmd(nc, in_maps, core_ids=list(range(NCORES)))
    yT = np.concatenate([res.results[k]["yT"] for k in range(NCORES)], axis=1)
    return np.ascontiguousarray(yT.T).astype(np.float32)


# revision 8
# speedup vs baseline: 1.2344x; 1.2344x over previous
"""LRU (complex diagonal linear recurrence, fwd+bwd) on 8 TRN2 NeuronCores.

Algorithm (validated in numpy): sequence-parallel over T. Per core:
  Bu^T = B_norm @ x_chunk^T  (fp32r matmuls)
  rotation trick: w = e^{-i*theta*tau} (.) Bu  -> complex scan becomes two
  real first-order scans with multiplier r (hardware tensor_tensor_scan)
  cross-core carries via AllGather of chunk-end states; correction applied
  in v-space as a single scalar_tensor_tensor per component (real decay)
  s = e^{+i*theta*tau} (.) v ;  y^T = C-projections (fp16 matmuls) + D (.) x^T
Backward direction = same machinery on the time-reversed stream.
Host does all transposes/table precompute (free); device does all O(T*N) work.
"""

import numpy as np
from contextlib import ExitStack

import concourse.bass as bass
import concourse.tile as tile
from concourse import bacc, mybir
from concourse.bass_utils import run_bass_kernel_spmd

NCORES = 8
T, N, H = 16384, 512, 512
TC = T // NCORES          # 2048 timesteps per core
NT = N // 128             # 4 partition tiles of the state dim
HT = H // 128             # 4 partition tiles of the channel dim
KH = H // 128             # contraction subtiles for Bu matmul
F16 = mybir.dt.float16
F32 = mybir.dt.float32
F32R = mybir.dt.float32r
MUL = mybir.AluOpType.mult
ADD = mybir.AluOpType.add
SUB = mybir.AluOpType.subtract

_CACHE = {}


def _build_nc(profile=False):
    nc = bacc.Bacc(
        "TRN2", target_bir_lowering=False, debug=False,
        enable_asserts=False, num_devices=1 if profile else NCORES,
    )
    di = lambda n, s, d=F32: nc.dram_tensor(n, s, d, kind="ExternalInput")
    xT_d = di("xT", [H, TC], F16)
    BTre_d = di("BTre", [H, N], F16)
    BTim_d = di("BTim", [H, N], F16)
    cos_d = di("cosT", [N, TC], F16)
    sin_d = di("sinT", [N, TC], F16)
    rpw_d = di("rpow", [N, TC], F16)
    # consts columns: 0=r 1=ce 2=se 3=c1 4=s1 5=D
    cst_d = di("consts", [N, 8])
    CT_d = {(d_, c_): di(f"CT{d_}{c_}", [N, H], F16)
            for d_ in "fb" for c_ in "ri"}
    W_d = {(d_, c_): di(f"W{d_}{c_}", [N, 8]) for d_ in "fb" for c_ in "ri"}
    yT_d = nc.dram_tensor("yT", [H, TC], F32, kind="ExternalOutput")
    bin_d = nc.dram_tensor("ccin", [128, 16], F32)
    bout_d = nc.dram_tensor("ccout", [NCORES, 128, 16], F32)

    with tile.TileContext(nc) as tc, ExitStack() as ctx:
        pool = lambda name, bufs: ctx.enter_context(tc.tile_pool(name=name, bufs=bufs))
        p_xT = pool("xT", 4)
        p_BT = pool("BT", 8)
        p_tab = pool("tab", 4)          # cos/sin, transient per nt per phase
        p_rpw = pool("rpw", 2)
        p_cst = pool("cst", 4)
        p_CT = pool("CT", 16)
        p_bups = ctx.enter_context(tc.tile_pool(name="bups", bufs=2, space="PSUM"))
        p_bu16 = pool("bu16", 3)
        p_w = pool("w", 3)
        p_st = pool("st", 24)           # v tiles, s-hat tiles, rotation temps
        p_sm = pool("sm", 24)           # small (128,<=16) helpers
        p_ops = ctx.enter_context(tc.tile_pool(name="ops", bufs=3, space="PSUM"))
        p_yo = pool("yo", 3)

        # ---- resident loads ----
        xT_sb = []
        for h in range(HT):
            t_ = p_xT.tile([128, TC], F16, tag="xT")
            nc.sync.dma_start(t_[:], xT_d[h * 128:(h + 1) * 128, :])
            xT_sb.append(t_)
        BT_sb = {}
        for nm, dd in (("re", BTre_d), ("im", BTim_d)):
            for h in range(HT):
                t_ = p_BT.tile([128, N], F16, tag="BT")
                nc.sync.dma_start(t_[:], dd[h * 128:(h + 1) * 128, :])
                BT_sb[(nm, h)] = t_
        cst_sb = []
        for nt in range(NT):
            t_ = p_cst.tile([128, 8], F32, tag="cst")
            nc.sync.dma_start(t_[:], cst_d[nt * 128:(nt + 1) * 128, :])
            cst_sb.append(t_)
        CT_sb = {}
        for key, dd in CT_d.items():
            for nt in range(NT):
                t_ = p_CT.tile([128, H], F16, tag="CT")
                nc.sync.dma_start(t_[:], dd[nt * 128:(nt + 1) * 128, :])
                CT_sb[key + (nt,)] = t_
        W_sb = {}
        for key, dd in W_d.items():
            for nt in range(NT):
                t_ = p_sm.tile([128, 8], F32, tag="sm")
                nc.sync.dma_start(t_[:], dd[nt * 128:(nt + 1) * 128, :])
                W_sb[key + (nt,)] = t_

        # ---- per N-tile: Bu matmuls, pre-rotations, pass-1 scans ----
        v_sb = {}      # (nt, dir, comp) -> fp16 (128, TC) local-scan outputs
        epk = p_sm.tile([128, 16], F32, tag="epk")   # packed end states
        for nt in range(NT):
            cos_t = p_tab.tile([128, TC], F16, tag="tab")
            nc.sync.dma_start(cos_t[:], cos_d[nt * 128:(nt + 1) * 128, :])
            sin_t = p_tab.tile([128, TC], F16, tag="tab")
            nc.sync.dma_start(sin_t[:], sin_d[nt * 128:(nt + 1) * 128, :])
            bu16 = {}
            for ci, nm in enumerate(("re", "im")):
                bu = p_bu16.tile([128, TC], F16, tag="bu16")
                for half in range(2):
                    ps = p_bups.tile([128, TC // 2], F32, tag="bups")
                    for lc in range(2):
                        sl = slice(half * 1024 + lc * 512, half * 1024 + (lc + 1) * 512)
                        psl = slice(lc * 512, (lc + 1) * 512)
                        for kh in range(KH):
                            nc.tensor.matmul(
                                ps[:, psl],
                                BT_sb[(nm, kh)][:, nt * 128:(nt + 1) * 128],
                                xT_sb[kh][:, sl],
                                start=(kh == 0), stop=(kh == KH - 1),
                            )
                    nc.scalar.copy(bu[:, half * 1024:(half + 1) * 1024], ps[:])
                bu16[nm] = bu
            rbc = cst_sb[nt][:, 0:1].broadcast_to([128, TC])
            for d_ in "fb":
                if d_ == "f":
                    a = bu16["re"][:]; b = bu16["im"][:]
                else:
                    a = bu16["re"][:, ::-1]; b = bu16["im"][:, ::-1]
                t1 = p_st.tile([128, TC], F16, tag="st")
                t2 = p_st.tile([128, TC], F16, tag="st")
                t3 = p_st.tile([128, TC], F16, tag="st")
                t4 = p_st.tile([128, TC], F16, tag="st")
                nc.vector.tensor_tensor(t1[:], cos_t[:], a, MUL)
                nc.vector.tensor_tensor(t2[:], sin_t[:], b, MUL)
                nc.vector.tensor_tensor(t3[:], cos_t[:], b, MUL)
                nc.vector.tensor_tensor(t4[:], sin_t[:], a, MUL)
                w_re = p_w.tile([128, TC], F16, tag="w")
                nc.vector.tensor_tensor(w_re[:], t1[:], t2[:], ADD)
                w_im = p_w.tile([128, TC], F16, tag="w")
                nc.vector.tensor_tensor(w_im[:], t3[:], t4[:], SUB)
                for ci, wt in (("re", w_re), ("im", w_im)):
                    v = p_st.tile([128, TC], F16, tag="st")
                    nc.vector.tensor_tensor_scan(v[:], rbc, wt[:], 0.0, MUL, ADD)
                    v_sb[(nt, d_, ci)] = v
                # end states -> s-space: E = (ce + i*se) * v_end
                ce = cst_sb[nt][:, 1:2]; se = cst_sb[nt][:, 2:3]
                vre = v_sb[(nt, d_, "re")][:, TC - 1:TC]
                vim = v_sb[(nt, d_, "im")][:, TC - 1:TC]
                tt = p_sm.tile([128, 1], F32, tag="sm")
                col = (0 if d_ == "f" else 8) + nt * 2
                nc.vector.tensor_scalar_mul(tt[:], vim, se)
                nc.vector.scalar_tensor_tensor(epk[:, col:col + 1], vre, ce, tt[:], MUL, SUB)
                nc.vector.tensor_scalar_mul(tt[:], vre, se)
                nc.vector.scalar_tensor_tensor(epk[:, col + 1:col + 2], vim, ce, tt[:], MUL, ADD)

        # ---- carry exchange ----
        nc.sync.dma_start(bin_d[:, :], epk[:])
        if profile:
            # TimelineSim can't model collectives; stand in a same-cost DMA
            for j in range(NCORES):
                nc.sync.dma_start(bout_d.ap()[j, :, :], bin_d[:, :])
        else:
            nc.gpsimd.collective_compute(
                "AllGather", mybir.AluOpType.bypass,
                replica_groups=[list(range(NCORES))],
                ins=[bin_d.ap().opt()], outs=[bout_d.ap().opt()],
            )
        chv = {}
        for d_ in "fb":
            for nt in range(NT):
                col = (0 if d_ == "f" else 8) + nt * 2
                eg = p_sm.tile([128, 16], F32, tag="eg")
                nc.sync.dma_start(
                    eg[:].rearrange("p (j c) -> p j c", c=2),
                    bout_d.ap()[:, :, col:col + 2].rearrange("j p c -> p j c"),
                )
                er = eg[:, 0:16:2]; ei = eg[:, 1:16:2]
                wre = W_sb[(d_, "r", nt)][:]; wim = W_sb[(d_, "i", nt)][:]
                pr = p_sm.tile([128, 8], F32, tag="pr")
                pi = p_sm.tile([128, 8], F32, tag="pr")
                cre = p_sm.tile([128, 1], F32, tag="cc")
                cim = p_sm.tile([128, 1], F32, tag="cc")
                nc.vector.tensor_tensor(pr[:], wre, er, MUL)
                nc.vector.tensor_tensor(pi[:], wim, ei, MUL)
                nc.vector.tensor_tensor(pr[:], pr[:], pi[:], SUB)
                nc.vector.tensor_reduce(cre[:], pr[:], mybir.AxisListType.X, ADD)
                nc.vector.tensor_tensor(pr[:], wre, ei, MUL)
                nc.vector.tensor_tensor(pi[:], wim, er, MUL)
                nc.vector.tensor_tensor(pr[:], pr[:], pi[:], ADD)
                nc.vector.tensor_reduce(cim[:], pr[:], mybir.AxisListType.X, ADD)
                # chv = e^{i theta} * c
                c1 = cst_sb[nt][:, 3:4]; s1 = cst_sb[nt][:, 4:5]
                tt = p_sm.tile([128, 1], F32, tag="sm")
                vr = p_sm.tile([128, 1], F32, tag="cv")
                vi = p_sm.tile([128, 1], F32, tag="cv")
                nc.vector.tensor_scalar_mul(tt[:], cim[:], s1)
                nc.vector.scalar_tensor_tensor(vr[:], cre[:], c1, tt[:], MUL, SUB)
                nc.vector.tensor_scalar_mul(tt[:], cre[:], s1)
                nc.vector.scalar_tensor_tensor(vi[:], cim[:], c1, tt[:], MUL, ADD)
                chv[(nt, d_, "re")] = vr
                chv[(nt, d_, "im")] = vi

        # ---- corrections + post-rotations ----
        sh_sb = {}
        for nt in range(NT):
            rpw = p_rpw.tile([128, TC], F16, tag="rpw")
            nc.sync.dma_start(rpw[:], rpw_d[nt * 128:(nt + 1) * 128, :])
            cos_t = p_tab.tile([128, TC], F16, tag="tab")
            nc.sync.dma_start(cos_t[:], cos_d[nt * 128:(nt + 1) * 128, :])
            sin_t = p_tab.tile([128, TC], F16, tag="tab")
            nc.sync.dma_start(sin_t[:], sin_d[nt * 128:(nt + 1) * 128, :])
            for d_ in "fb":
                vt = {}
                for ci in ("re", "im"):
                    v2 = p_st.tile([128, TC], F16, tag="st")
                    nc.vector.scalar_tensor_tensor(
                        v2[:], rpw[:], chv[(nt, d_, ci)][:],
                        v_sb[(nt, d_, ci)][:], MUL, ADD)
                    vt[ci] = v2
                t1 = p_st.tile([128, TC], F16, tag="st")
                t2 = p_st.tile([128, TC], F16, tag="st")
                t3 = p_st.tile([128, TC], F16, tag="st")
                t4 = p_st.tile([128, TC], F16, tag="st")
                s_re = p_st.tile([128, TC], F16, tag="st")
                s_im = p_st.tile([128, TC], F16, tag="st")
                nc.vector.tensor_tensor(t1[:], sin_t[:], vt["re"][:], MUL)
                nc.vector.tensor_tensor(t2[:], cos_t[:], vt["im"][:], MUL)
                nc.vector.tensor_tensor(s_im[:] if d_ == "f" else s_im[:, ::-1],
                                        t1[:], t2[:], ADD)
                nc.vector.tensor_tensor(t3[:], cos_t[:], vt["re"][:], MUL)
                nc.vector.tensor_tensor(t4[:], sin_t[:], vt["im"][:], MUL)
                nc.vector.tensor_tensor(s_re[:] if d_ == "f" else s_re[:, ::-1],
                                        t3[:], t4[:], SUB)
                sh_sb[(nt, d_, "re")] = s_re
                sh_sb[(nt, d_, "im")] = s_im

        # ---- output matmuls + D term ----
        for lc in range(4):
            lsl = slice(lc * 512, (lc + 1) * 512)
            for ht in range(HT):
                ps = p_ops.tile([128, 512], F32, tag="ops")
                groups = [(d_, c_, nt) for d_ in "fb" for c_ in "ri"
                          for nt in range(NT)]
                for gi, (d_, c_, nt) in enumerate(groups):
                    nc.tensor.matmul(
                        ps[:],
                        CT_sb[(d_, c_, nt)][:, ht * 128:(ht + 1) * 128],
                        sh_sb[(nt, d_, "re" if c_ == "r" else "im")][:, lsl],
                        start=(gi == 0), stop=(gi == len(groups) - 1),
                    )
                yo = p_yo.tile([128, 512], F32, tag="yo")
                nc.vector.scalar_tensor_tensor(
                    yo[:], xT_sb[ht][:, lsl], cst_sb[ht][:, 5:6], ps[:], MUL, ADD)
                nc.sync.dma_start(yT_d[ht * 128:(ht + 1) * 128, lsl], yo[:])

    nc.compile()
    return nc


def _host_prep(x, theta_log, nu_log, B_re, B_im, C_re, C_im, C_re2, C_im2, D):
    f64 = np.float64
    theta = np.exp(theta_log.astype(f64))
    r = np.exp(-np.exp(nu_log.astype(f64)))
    gamma = np.sqrt(1.0 - r ** 2)
    Bn = (B_re.astype(f64) + 1j * B_im.astype(f64)) * gamma[:, None]
    Lam = r * np.exp(1j * theta)
    tau = np.arange(TC, dtype=f64)
    cosT = np.cos(theta[:, None] * tau).astype(np.float16)
    sinT = np.sin(theta[:, None] * tau).astype(np.float16)
    rpow = (r[:, None] ** (tau + 1)).astype(np.float16)
    consts = np.zeros((N, 8), np.float32)
    consts[:, 0] = r
    consts[:, 1] = np.cos(theta * (TC - 1)); consts[:, 2] = np.sin(theta * (TC - 1))
    consts[:, 3] = np.cos(theta); consts[:, 4] = np.sin(theta)
    consts[:, 5] = D
    xT = np.ascontiguousarray(x.T.astype(np.float16))        # (H, T)
    BTre = np.ascontiguousarray(Bn.real.T.astype(np.float16))
    BTim = np.ascontiguousarray(Bn.imag.T.astype(np.float16))
    C1 = C_re.astype(f64) + 1j * C_im.astype(f64)
    C2 = C_re2.astype(f64) + 1j * C_im2.astype(f64)
    CT = {
        ("f", "r"): C1.real.T, ("f", "i"): -C1.imag.T,
        ("b", "r"): C2.real.T, ("b", "i"): -C2.imag.T,
    }
    CT = {k: np.ascontiguousarray(v.astype(np.float16)) for k, v in CT.items()}
    LamTC = Lam ** TC
    W = {}
    for k in range(NCORES):
        wf = np.zeros((N, 8), np.complex128)
        wb = np.zeros((N, 8), np.complex128)
        for j in range(k):
            wf[:, j] = LamTC ** (k - 1 - j)
        for j in range(k + 1, NCORES):
            wb[:, j] = LamTC ** (j - k - 1)
        W[k] = (wf, wb)
    return xT, BTre, BTim, cosT, sinT, rpow, consts, CT, W


def kernel(**inputs):
    if "nc" not in _CACHE:
        _CACHE["nc"] = _build_nc()
    nc = _CACHE["nc"]
    xT, BTre, BTim, cosT, sinT, rpow, consts, CT, W = _host_prep(**inputs)
    in_maps = []
    for k in range(NCORES):
        wf, wb = W[k]
        m = {
            "xT": np.ascontiguousarray(xT[:, k * TC:(k + 1) * TC]),
            "BTre": BTre, "BTim": BTim,
            "cosT": cosT, "sinT": sinT, "rpow": rpow, "consts": consts,
            "CTfr": CT[("f", "r")], "CTfi": CT[("f", "i")],
            "CTbr": CT[("b", "r")], "CTbi": CT[("b", "i")],
            "Wfr": np.ascontiguousarray(wf.real.astype(np.float32)),
            "Wfi": np.ascontiguousarray(wf.imag.astype(np.float32)),
            "Wbr": np.ascontiguousarray(wb.real.astype(np.float32)),
            "Wbi": np.ascontiguousarray(wb.imag.astype(np.float32)),
        }
        in_maps.append(m)
    res = run_bass_kernel_spmd(nc, in_maps, core_ids=list(range(NCORES)))
    yT = np.concatenate([res.results[k]["yT"] for k in range(NCORES)], axis=1)
    return np.ascontiguousarray(yT.T).astype(np.float32)
